# revision 53
# baseline (speedup 1.0000x reference)
"""Trainium2 Bass kernel for nn_Dytrans_45561013076435 (Informer ProbSparse block).

Sharding: 8 cores = 4 batches x 2 head-groups (4 heads each). Core c
handles b=c//2, heads [4g..4g+4) with g=c%2, then the FFN for l-half g
after a tiny fp16 AllGather of per-head (upd40, l40, vmean) in each pair.

Selection: approx M via fp32r qk with the fp8e5 sample-mask added on the
PE (identity-matmul into the PSUM accumulation group), row-max via a
bf16 2x pairwise-max cascade on DVE; mean via fp8 DoubleRow A@k;
overselect top-64; exact recheck via bf16 hi/lo split-3 matmuls; top-40
of 64 via gpsimd kth_largest + sparse_gather.

Post phase: ctx carries only scattered (upd - vmean) deltas into a
pre-zeroed DRAM buffer; the rank-1 vmean baseline (sum_h Wo_h^T vmean_h)
is folded into the P7 residual as a per-partition scalar. FFN weights
ship pre-interleaved for fp8 DoubleRow matmuls (2 contraction chunks
per pass); FFN runs fully in fp8. FFN/Wo loads are issued before the
AllGather so they hide under its fixed latency.
"""
import sys
sys.path.insert(0, "/opt/trn_rl_repo")
from contextlib import ExitStack
import numpy as np
import ml_dtypes

import concourse.bass as bass
import concourse.bacc as bacc
import concourse.mybir as mybir
import concourse.tile as tile
from concourse.bass_utils import run_bass_kernel_spmd

dt = mybir.dt
F32, F32R, BF16, FP16 = dt.float32, dt.float32r, dt.bfloat16, dt.float16
FP8 = dt.float8e4
FP8E5 = dt.float8e5
U32, U8 = dt.uint32, dt.uint8
ALU = mybir.AluOpType
AF = mybir.ActivationFunctionType
IOA = bass.IndirectOffsetOnAxis
AXX = mybir.AxisListType.X

L, B, C, H, D, S, EXP = 2048, 4, 512, 8, 64, 40, 4
NLT = L // 128
HL = L // 2
CSEL = 64
Q1 = 1.0 - 63.5 / 2047.0
Q2 = 1.0 - 39.5 / 63.0
NEG = -30000.0
SEG = S * D + 64 + 64
bfd = ml_dtypes.bfloat16
f16 = np.float16

_NC_CACHE = {}


def _kernel(nc, tc):
    P = 128
    ctx = ExitStack()
    # ---------------- dram I/O ----------------
    qTh = nc.dram_tensor("qTh", [C, L], F32R, kind="ExternalInput").ap()
    qThalf = nc.dram_tensor("qThalf", [C, HL], F32, kind="ExternalInput").ap()
    WqT = nc.dram_tensor("WqT", [C, 256], F32R, kind="ExternalInput").ap()
    WkT = nc.dram_tensor("WkT", [C, 256], F32R, kind="ExternalInput").ap()
    WvT = nc.dram_tensor("WvT", [C, 256], F32R, kind="ExternalInput").ap()
    WqTh = nc.dram_tensor("WqTh", [C, 256], BF16, kind="ExternalInput").ap()
    WqTl = nc.dram_tensor("WqTl", [C, 256], BF16, kind="ExternalInput").ap()
    WkTh = nc.dram_tensor("WkTh", [C, 256], BF16, kind="ExternalInput").ap()
    WkTl = nc.dram_tensor("WkTl", [C, 256], BF16, kind="ExternalInput").ap()
    Wo8 = nc.dram_tensor("Wo8", [H * D, C], FP16, kind="ExternalInput").ap()
    W1DR = [nc.dram_tensor(f"W1DR{p_}", [P, 2 * EXP * C], FP8,
                           kind="ExternalInput").ap() for p_ in range(2)]
    W2DR = [nc.dram_tensor(f"W2DR{pp}", [P, 2 * C], FP8,
                           kind="ExternalInput").ap() for pp in range(8)]
    maskb = nc.dram_tensor("maskb", [L, L], FP8E5, kind="ExternalInput").ap()
    ATDR = [nc.dram_tensor(f"ATDR{pp}", [P, 2 * L], FP8,
                           kind="ExternalInput").ap() for pp in range(8)]
    Ab = nc.dram_tensor("Ab", [L, L], FP8, kind="ExternalInput").ap()
    n1a = nc.dram_tensor("n1a", [P, 1], F32, kind="ExternalInput").ap()
    n1w = nc.dram_tensor("n1w", [C, 1], F32, kind="ExternalInput").ap()
    n2a = nc.dram_tensor("n2a", [P, 1], F32, kind="ExternalInput").ap()
    n2w = nc.dram_tensor("n2w", [C, 1], F32, kind="ExternalInput").ap()
    iot16 = nc.dram_tensor("iot16", [P, 16], F32, kind="ExternalInput").ap()
    iot64 = nc.dram_tensor("iot64", [CSEL], F32, kind="ExternalInput").ap()
    ones2 = nc.dram_tensor("ones2", [P, 2], F32R, kind="ExternalInput").ap()
    gshift = nc.dram_tensor("gshift", [P, 1], F32, kind="ExternalInput").ap()
    identb = nc.dram_tensor("identb", [P, P], BF16, kind="ExternalInput").ap()
    oout = nc.dram_tensor("oout", [C, HL], F32, kind="ExternalOutput").ap()

    # ---------------- internal dram ----------------
    xnTd = [nc.dram_tensor(f"xnTd{ct}", [128, L], F32).ap() for ct in range(4)]
    thrd = nc.dram_tensor("thrd", [4], F32).ap()
    thrd2 = nc.dram_tensor("thrd2", [4], F32).ap()
    band = nc.dram_tensor("band", [4, L], F32).ap()
    candd = nc.dram_tensor("candd", [4, CSEL], F32).ap()
    bl2 = nc.dram_tensor("bl2", [4, CSEL], F32).ap()
    bp2 = nc.dram_tensor("bp2", [4, CSEL], F32).ap()
    sld = nc.dram_tensor("sld", [4, CSEL], F32).ap()
    spd = nc.dram_tensor("spd", [4, CSEL], F32).ap()
    upd64d = [nc.dram_tensor(f"upd64d{h}", [CSEL, D], FP16).ap()
              for h in range(4)]
    exin = nc.dram_tensor("exin", [4 * SEG], FP16).ap()
    exout = nc.dram_tensor("exout", [8 * SEG], FP16).ap()
    ctxd = [nc.dram_tensor(f"ctxd{h8}", [HL, D], FP16).ap() for h8 in range(8)]
    zrowd = nc.dram_tensor("zrowd", [1, D], FP16).ap()

    # ---------------- whole-kernel consts ----------------
    consts = ctx.enter_context(tc.tile_pool(name="consts", bufs=1))
    n1a_t = consts.tile([P, 1], F32); nc.sync.dma_start(n1a_t[:], n1a[:, :])
    n2a_t = consts.tile([P, 1], F32); nc.sync.dma_start(n2a_t[:], n2a[:, :])
    n1w_t = consts.tile([P, 4], F32)
    nc.sync.dma_start(n1w_t[:], n1w.rearrange("(ct p) o -> p (ct o)", p=128))
    n2w_t = consts.tile([P, 4], F32)
    nc.sync.dma_start(n2w_t[:], n2w.rearrange("(ct p) o -> p (ct o)", p=128))
    iot16_t = consts.tile([P, 16], F32); nc.sync.dma_start(iot16_t[:], iot16[:, :])
    iot64_t = consts.tile([CSEL, 1], F32)
    nc.sync.dma_start(iot64_t[:], iot64.rearrange("(p o) -> p o", o=1))
    ones_f32 = consts.tile([P, 2], F32)
    nc.sync.dma_start(ones_f32[:], ones2[:, :].bitcast(F32))
    ones_c2f = consts.tile([P, 2], FP16)
    nc.vector.tensor_copy(ones_c2f[:], ones_f32[:])
    gsh_t = consts.tile([P, 1], F32); nc.sync.dma_start(gsh_t[:], gshift[:, :])
    neg16 = consts.tile([P, 16], F32); nc.vector.memset(neg16[:], -1.0)
    idb = consts.tile([P, P], BF16); nc.sync.dma_start(idb[:], identb[:, :])
    id8 = consts.tile([P, P], FP8E5); nc.vector.tensor_copy(id8[:], idb[:])
    zrow = consts.tile([1, D], FP16); nc.vector.memset(zrow[:], 0.0)
    neg64 = consts.tile([CSEL, 1], F32); nc.vector.memset(neg64[:], -1.0)
    Mmax = [consts.tile([P, 16], F32, tag=f"Mmax{h}", name=f"Mmax{h}") for h in range(4)]
    Mmean = [consts.tile([P, 16], F32, tag=f"Mmean{h}", name=f"Mmean{h}") for h in range(4)]
    Msb = [consts.tile([P, 16], F32, tag=f"Msb{h}", name=f"Msb{h}") for h in range(4)]
    # ctx holds only scattered (upd - vmean) deltas; zero it early, off the
    # critical path (vmean baseline is folded into the P7 residual as rank-1)
    nc.gpsimd.dma_start(zrowd[:, :], zrow[0:1, :])
    for h8 in range(8):
        nc.gpsimd.dma_start(
            ctxd[h8][:, :],
            zrowd[:, :].rearrange("o d -> o d").to_broadcast([HL, D]))

    # =================== attention phases (big SBUF scope) ===================
    with tc.tile_pool(name="bpool", bufs=1) as bpool:
        xn = [bpool.tile([P, L], F32R, tag=f"xn{ct}", name=f"xn{ct}") for ct in range(4)]
        xnh = [bpool.tile([P, L], BF16, tag=f"xnh{ct}", name=f"xnh{ct}") for ct in range(4)]
        xnl = [bpool.tile([P, L], BF16, tag=f"xnl{ct}", name=f"xnl{ct}") for ct in range(4)]
        at8 = [bpool.tile([P, 2 * L], FP8, tag=f"at{pp}", name=f"at{pp}")
               for pp in range(8)]
        vall = bpool.tile([P, NLT * 256], FP16)
        klc8 = bpool.tile([P, NLT * 256], FP8)
        wq = [bpool.tile([P, 256], F32R, tag=f"wq{ct}", name=f"wq{ct}") for ct in range(4)]
        wk = [bpool.tile([P, 256], F32R, tag=f"wk{ct}", name=f"wk{ct}") for ct in range(4)]
        wv = [bpool.tile([P, 256], F32R, tag=f"wv{ct}", name=f"wv{ct}") for ct in range(4)]
        wqh = [bpool.tile([P, 256], BF16, tag=f"wqh{ct}", name=f"wqh{ct}") for ct in range(4)]
        wql = [bpool.tile([P, 256], BF16, tag=f"wql{ct}", name=f"wql{ct}") for ct in range(4)]
        wkh = [bpool.tile([P, 256], BF16, tag=f"wkh{ct}", name=f"wkh{ct}") for ct in range(4)]
        wkl = [bpool.tile([P, 256], BF16, tag=f"wkl{ct}", name=f"wkl{ct}") for ct in range(4)]
        for ct in range(4):
            nc.sync.dma_start(wq[ct][:], WqT[128 * ct:128 * (ct + 1), :])
            nc.sync.dma_start(wk[ct][:], WkT[128 * ct:128 * (ct + 1), :])
            nc.sync.dma_start(wv[ct][:], WvT[128 * ct:128 * (ct + 1), :])

        # -------- P1: norm1 --------
        with tc.tile_pool(name="qstream", bufs=2) as qstream:
            for ct in range(4):
                tmp = qstream.tile([P, L], F32, tag="xntmp")
                for hf in range(2):
                    sl = slice(1024 * hf, 1024 * (hf + 1))
                    qt = qstream.tile([P, 1024], F32, tag="query")
                    nc.sync.dma_start(qt[:],
                                      qTh[128 * ct:128 * (ct + 1), sl].bitcast(F32))
                    sig = qstream.tile([P, 1024], F32, tag="sig")
                    nc.scalar.activation(sig[:], qt[:], AF.Sigmoid,
                                         scale=n1a_t[:, :])
                    nc.vector.scalar_tensor_tensor(
                        out=tmp[:, sl], in0=sig[:],
                        scalar=n1w_t[:, ct:ct + 1], in1=qt[:],
                        op0=ALU.mult, op1=ALU.mult)
                nc.scalar.copy(xn[ct][:], tmp[:])          # round to fp32r
                nc.scalar.copy(xnh[ct][:], tmp[:])
                nc.vector.tensor_tensor(out=xnl[ct][:], in0=tmp[:],
                                        in1=xnh[ct][:], op=ALU.subtract)
                # off the SP queue: input loads saturate it at kernel start
                nc.gpsimd.dma_start(xnTd[ct][:, :], tmp[:])

        for ct in range(4):
            nc.sync.dma_start(wqh[ct][:], WqTh[128 * ct:128 * (ct + 1), :])
            nc.sync.dma_start(wql[ct][:], WqTl[128 * ct:128 * (ct + 1), :])
            nc.sync.dma_start(wkh[ct][:], WkTh[128 * ct:128 * (ct + 1), :])
            nc.sync.dma_start(wkl[ct][:], WkTl[128 * ct:128 * (ct + 1), :])
        for pp in range(8):
            nc.gpsimd.dma_start(at8[pp][:], ATDR[pp][:, :])

        # -------- P2 + P3 --------
        with tc.tile_pool(name="qkpool", bufs=1) as qkpool:
            qTs = [qkpool.tile([P, L], F32R, tag=f"qT{pk}", name=f"qT{pk}") for pk in range(2)]
            kTs = [qkpool.tile([P, L], F32R, tag=f"kT{pk}", name=f"kT{pk}") for pk in range(2)]
            with tc.tile_pool(name="p2ps", bufs=2, space="PSUM") as p2ps:
                for pk in range(2):
                    for dst, w in ((qTs[pk], wq), (kTs[pk], wk)):
                        for nck in range(4):
                            ps = p2ps.tile([P, 512], F32, tag="proj")
                            for kt in range(4):
                                nc.tensor.matmul(
                                    ps[:], w[kt][:, 128 * pk:128 * (pk + 1)],
                                    xn[kt][:, 512 * nck:512 * (nck + 1)],
                                    start=(kt == 0), stop=(kt == 3))
                            nc.scalar.copy(dst[:, 512 * nck:512 * (nck + 1)], ps[:])
                for lt in range(NLT):
                    for which, w in (("k", wk), ("v", wv)):
                        ps = p2ps.tile([P, 256], F32, tag="lc")
                        for kt in range(4):
                            nc.tensor.matmul(
                                ps[:],
                                xn[kt][:, 128 * lt:128 * (lt + 1)],
                                w[kt][:], start=(kt == 0), stop=(kt == 3))
                        if which == "k":
                            nc.scalar.copy(klc8[:, 256 * lt:256 * (lt + 1)], ps[:])
                        else:
                            nc.scalar.copy(vall[:, 256 * lt:256 * (lt + 1)], ps[:])

            with tc.tile_pool(name="p3", bufs=2) as p3, \
                 tc.tile_pool(name="p3ps", bufs=2, space="PSUM") as p3ps:
                for lt in range(NLT):
                    qlcp = p3ps.tile([P, 256], F32, tag="lc")
                    for kt in range(4):
                        nc.tensor.matmul(
                            qlcp[:], xn[kt][:, 128 * lt:128 * (lt + 1)],
                            wq[kt][:], start=(kt == 0), stop=(kt == 3))
                    qlc = p3.tile([P, 256], F32, tag="qlc")
                    nc.scalar.copy(qlc[:], qlcp[:])
                    ksum = p3ps.tile([P, 256], F32, tag="ksum")
                    for pp in range(8):
                        nc.tensor.matmul(
                            ksum[:],
                            at8[pp][:, 256 * lt:256 * (lt + 1)]
                            .rearrange("p (two c) -> p two c", two=2),
                            klc8[:, 512 * pp:512 * (pp + 1)]
                            .rearrange("p (two f) -> p two f", two=2),
                            start=(pp == 0), stop=(pp == 7),
                            perf_mode=mybir.MatmulPerfMode.DoubleRow)
                    for h in range(4):
                        msc = p3.tile([P, D], F32, tag="msc")
                        nc.vector.scalar_tensor_tensor(
                            out=msc[:], in0=ksum[:, 64 * h:64 * (h + 1)],
                            scalar=1.0, in1=qlc[:, 64 * h:64 * (h + 1)],
                            op0=ALU.mult, op1=ALU.mult,
                            accum_out=Mmean[h][:, lt:lt + 1])
                    mt = p3.tile([P, L], FP8E5, tag="mask")
                    nc.gpsimd.dma_start(mt[:], maskb[128 * lt:128 * (lt + 1), :])
                    for h in range(4):
                        pk, hh = h // 2, h % 2
                        qksb = p3.tile([P, L], BF16, tag="qksb")
                        # mask-add via PE identity-matmul into PSUM (fp8e5
                        # mask rides the accumulation group at full rate)
                        for ck in range(2):
                            qk = p3ps.tile([P, 1024], F32, tag="qk")
                            for nk in range(2):
                                off = 1024 * ck + 512 * nk
                                nc.tensor.matmul(
                                    qk[:, 512 * nk:512 * (nk + 1)],
                                    qTs[pk][64 * hh:64 * (hh + 1),
                                            128 * lt:128 * (lt + 1)],
                                    kTs[pk][64 * hh:64 * (hh + 1), off:off + 512],
                                    start=True, stop=False)
                                nc.tensor.matmul(
                                    qk[:, 512 * nk:512 * (nk + 1)],
                                    id8[:], mt[:, off:off + 512],
                                    start=False, stop=True)
                            nc.scalar.copy(qksb[:, 1024 * ck:1024 * (ck + 1)],
                                           qk[:])
                        scm = p3.tile([P, 1024], BF16, tag="scm")
                        nc.vector.tensor_tensor(out=scm[:], in0=qksb[:, 0:1024],
                                                in1=qksb[:, 1024:2048],
                                                op=ALU.max)
                        nc.vector.tensor_tensor(out=scm[:, 0:512],
                                                in0=scm[:, 0:512],
                                                in1=scm[:, 512:1024],
                                                op=ALU.max)
                        nc.vector.tensor_tensor(out=scm[:, 0:256],
                                                in0=scm[:, 0:256],
                                                in1=scm[:, 256:512],
                                                op=ALU.max)
                        nc.vector.tensor_reduce(out=Mmax[h][:, lt:lt + 1],
                                                in_=scm[:, 0:256], axis=AXX,
                                                op=ALU.max)
                for h in range(4):
                    nc.vector.scalar_tensor_tensor(
                        out=Msb[h][:], in0=Mmean[h][:], scalar=-1.0 / S,
                        in1=Mmax[h][:], op0=ALU.mult, op1=ALU.add)

        # -------- P4: stage-1 top-64 --------
        with tc.tile_pool(name="sel", bufs=1) as sel:
            offs64 = [sel.tile([CSEL, 1], U32, tag=f"o64_{h}", name=f"o64_{h}") for h in range(4)]
            candf = [sel.tile([CSEL, 1], F32, tag=f"cf{h}", name=f"cf{h}") for h in range(4)]
            with tc.tile_pool(name="selw", bufs=3) as selw:
                for h in range(4):
                    thr = selw.tile([1, 2], F32, tag="thr")
                    nc.gpsimd.kth_largest(thr[:], Msb[h][:], n_per_lane=16, k=72,
                                          quantile=Q1)
                    thrb = selw.tile([P, 1], F32, tag="thrb")
                    nc.gpsimd.partition_broadcast(thrb[:], thr[0:1, 0:1])
                    selm = selw.tile([P, 16], U8, tag="selm")
                    nc.vector.tensor_scalar(out=selm[:], in0=Msb[h][:],
                                            scalar1=thrb[:, :], scalar2=None,
                                            op0=ALU.is_gt)
                    cand = selw.tile([P, 16], F32, tag="cand")
                    nc.vector.select(cand[:], selm[:], iot16_t[:], neg16[:])
                    nc.sync.dma_start(band[h, :].rearrange("(t p) -> p t", p=128),
                                      cand[:, :])
                    c16 = selw.tile([16, 128], F32, tag="c16")
                    nc.sync.dma_start(c16[:],
                                      band[h, :].rearrange("(f p) -> p f", p=16))
                    sg = selw.tile([16, 4], F32, tag="sg")
                    nf = selw.tile([1, 1], U32, tag="nf")
                    nc.vector.memset(sg[:], 0.0)
                    nc.gpsimd.sparse_gather(sg[:], c16[:], num_found=nf[:])
                    nc.sync.dma_start(candd[h, :].rearrange("(f p) -> p f", p=16),
                                      sg[:, :])
                    nc.sync.dma_start(candf[h][:],
                                      candd[h, :].rearrange("(p o) -> p o", o=1))
                    nc.vector.tensor_copy(offs64[h][:], candf[h][:])

            # -------- P5: exact recheck + attention smalls --------
            with tc.tile_pool(name="expool", bufs=1) as expool, \
                 tc.tile_pool(name="exps", bufs=2, space="PSUM") as exps, \
                 tc.tile_pool(name="exps1", bufs=1, space="PSUM") as exps1:
                kxh = [expool.tile([P, L], BF16, tag=f"kxh{pk}", name=f"kxh{pk}") for pk in range(2)]
                kxl = [expool.tile([P, L], BF16, tag=f"kxl{pk}", name=f"kxl{pk}") for pk in range(2)]
                with tc.tile_pool(name="exw", bufs=2) as exw:
                    for pk in range(2):
                        for nck in range(4):
                            ps = exps.tile([P, 512], F32, tag="ps512")
                            mmi = 0
                            for kt in range(4):
                                for a, b_ in ((xnh, wkh), (xnh, wkl), (xnl, wkh)):
                                    nc.tensor.matmul(
                                        ps[:], b_[kt][:, 128 * pk:128 * (pk + 1)],
                                        a[kt][:, 512 * nck:512 * (nck + 1)],
                                        start=(mmi == 0), stop=(mmi == 11))
                                    mmi += 1
                            nc.scalar.copy(kxh[pk][:, 512 * nck:512 * (nck + 1)],
                                           ps[:])
                            nc.vector.tensor_tensor(
                                out=kxl[pk][:, 512 * nck:512 * (nck + 1)],
                                in0=ps[:],
                                in1=kxh[pk][:, 512 * nck:512 * (nck + 1)],
                                op=ALU.subtract)

                    for h in range(4):
                        pk, hh = h // 2, h % 2
                        x64h, x64l = [], []
                        for ct in range(4):
                            g = exw.tile([P, CSEL], F32, tag=f"g{ct}", name=f"g{ct}")
                            nc.gpsimd.indirect_dma_start(
                                out=g[:], out_offset=None,
                                in_=xnTd[ct][:, :],
                                in_offset=IOA(ap=offs64[h][:, 0:1], axis=1))
                            gh = exw.tile([P, CSEL], BF16, tag=f"gh{ct}", name=f"gh{ct}")
                            nc.scalar.copy(gh[:], g[:])
                            gl = exw.tile([P, CSEL], BF16, tag=f"gl{ct}", name=f"gl{ct}")
                            nc.vector.tensor_tensor(out=gl[:], in0=g[:], in1=gh[:],
                                                    op=ALU.subtract)
                            x64h.append(gh); x64l.append(gl)
                        q64p = exps.tile([D, CSEL], F32, tag="psS")
                        mmi = 0
                        for kt in range(4):
                            for a, b_ in ((x64h, wqh), (x64h, wql), (x64l, wqh)):
                                nc.tensor.matmul(
                                    q64p[:], b_[kt][:, 64 * h:64 * (h + 1)],
                                    a[kt][:], start=(mmi == 0), stop=(mmi == 11))
                                mmi += 1
                        q64h0 = exw.tile([D, CSEL], BF16, tag="q64h0")
                        nc.scalar.copy(q64h0[:], q64p[:])
                        q64l0 = exw.tile([D, CSEL], BF16, tag="q64l0")
                        nc.vector.tensor_tensor(out=q64l0[:], in0=q64p[:],
                                                in1=q64h0[:], op=ALU.subtract)
                        # shift to base partition 64*hh to match kx slices
                        q64sh = exw.tile([P, CSEL], BF16, tag="q64sh")
                        q64sl = exw.tile([P, CSEL], BF16, tag="q64sl")
                        nc.sync.dma_start(q64sh[64 * hh:64 * (hh + 1), :],
                                          q64h0[:, :])
                        nc.sync.dma_start(q64sl[64 * hh:64 * (hh + 1), :],
                                          q64l0[:, :])
                        q64h = q64sh[64 * hh:64 * (hh + 1), :]
                        q64l = q64sl[64 * hh:64 * (hh + 1), :]
                        m64 = exw.tile([CSEL, L], FP8E5, tag="m64")
                        nc.gpsimd.indirect_dma_start(
                            out=m64[:], out_offset=None, in_=maskb[:, :],
                            in_offset=IOA(ap=offs64[h][:, 0:1], axis=0))
                        a64 = exw.tile([CSEL, L], FP8, tag="a64")
                        nc.gpsimd.indirect_dma_start(
                            out=a64[:], out_offset=None, in_=Ab[:, :],
                            in_offset=IOA(ap=offs64[h][:, 0:1], axis=0))
                        mx4 = exw.tile([CSEL, 4], F32, tag="mx4")
                        mn4 = exw.tile([CSEL, 4], F32, tag="mn4")
                        for ckk in range(4):
                            csl = slice(512 * ckk, 512 * (ckk + 1))
                            qk64 = exps.tile([CSEL, 512], F32, tag="ps512")
                            mmi = 0
                            for a, b_ in ((q64h, kxh[pk]), (q64h, kxl[pk]),
                                          (q64l, kxh[pk])):
                                nc.tensor.matmul(
                                    qk64[:], a, b_[64 * hh:64 * (hh + 1), csl],
                                    start=(mmi == 0), stop=(mmi == 2))
                                mmi += 1
                            scm = exw.tile([CSEL, 512], F32, tag="scm")
                            nc.vector.scalar_tensor_tensor(
                                out=scm[:], in0=qk64[:], scalar=1.0,
                                in1=a64[:, csl], op0=ALU.mult, op1=ALU.mult,
                                accum_out=mn4[:, ckk:ckk + 1])
                            nc.vector.tensor_tensor(out=scm[:], in0=qk64[:],
                                                    in1=m64[:, csl], op=ALU.add)
                            nc.vector.tensor_reduce(out=mx4[:, ckk:ckk + 1],
                                                    in_=scm[:], axis=AXX, op=ALU.max)
                        m64max = exw.tile([CSEL, 1], F32, tag="m64max")
                        nc.vector.tensor_reduce(out=m64max[:], in_=mx4[:],
                                                axis=AXX, op=ALU.max)
                        m64mean = exw.tile([CSEL, 1], F32, tag="m64mean")
                        nc.vector.tensor_reduce(out=m64mean[:], in_=mn4[:],
                                                axis=AXX, op=ALU.add)
                        M64 = exw.tile([P, 1], F32, tag="M64")
                        nc.vector.memset(M64[:], -1e30)
                        nc.vector.scalar_tensor_tensor(
                            out=M64[0:CSEL, :], in0=m64mean[:], scalar=-1.0 / S,
                            in1=m64max[:], op0=ALU.mult, op1=ALU.add)
                        thr2 = exw.tile([1, 2], F32, tag="thr2")
                        nc.gpsimd.kth_largest(thr2[:], M64[:], n_per_lane=1, k=48,
                                              quantile=Q2)
                        thr2b = exw.tile([CSEL, 1], F32, tag="thr2b")
                        nc.gpsimd.partition_broadcast(thr2b[:], thr2[0:1, 0:1])
                        sm2 = exw.tile([CSEL, 1], U8, tag="sm2")
                        nc.vector.tensor_scalar(out=sm2[:], in0=M64[0:CSEL, :],
                                                scalar1=thr2b[:, :], scalar2=None,
                                                op0=ALU.is_gt)
                        sell = exw.tile([CSEL, 1], F32, tag="sell")
                        nc.vector.select(sell[:], sm2[:], candf[h][:], neg64[:])
                        selp = exw.tile([CSEL, 1], F32, tag="selp")
                        nc.vector.select(selp[:], sm2[:], iot64_t[:], neg64[:])
                        for src, dst_b, dst_s in ((sell, bl2, sld),
                                                  (selp, bp2, spd)):
                            nc.sync.dma_start(
                                dst_b[h, :].rearrange("(p o) -> p o", o=1),
                                src[:, :])
                            c16b = exw.tile([16, 4], F32, tag="c16b")
                            nc.sync.dma_start(
                                c16b[:], dst_b[h, :].rearrange("(f p) -> p f", p=16))
                            sg2 = exw.tile([16, 4], F32, tag="sg2")
                            nf2 = exw.tile([1, 1], U32, tag="nf2")
                            nc.vector.memset(sg2[:], 0.0)
                            nc.gpsimd.sparse_gather(sg2[:], c16b[:],
                                                    num_found=nf2[:])
                            nc.sync.dma_start(
                                dst_s[h, :].rearrange("(f p) -> p f", p=16),
                                sg2[:, :])
                        # softmax + upd over all 64 candidates
                        upd = exps1.tile([CSEL, D], F32, tag="upd")
                        usum = exps1.tile([CSEL, 2], F32, tag="usum")
                        for lt in range(NLT):
                            sTp = exps.tile([P, CSEL], F32, tag="psS")
                            mmi = 0
                            for a, b_ in ((kxh[pk], q64h), (kxh[pk], q64l),
                                          (kxl[pk], q64h)):
                                nc.tensor.matmul(
                                    sTp[:],
                                    a[64 * hh:64 * (hh + 1),
                                      128 * lt:128 * (lt + 1)],
                                    b_, start=(mmi == 0), stop=(mmi == 2))
                                mmi += 1
                            eT = exw.tile([P, CSEL], FP16, tag="eT")
                            nc.scalar.activation(eT[:], sTp[:], AF.Exp,
                                                 scale=1.0 / (D ** 0.5))
                            nc.tensor.matmul(
                                upd[:], eT[:],
                                vall[:, 256 * lt + 64 * h:256 * lt + 64 * (h + 1)],
                                start=(lt == 0), stop=(lt == NLT - 1))
                            nc.tensor.matmul(usum[:], eT[:], ones_c2f[:],
                                             start=(lt == 0), stop=(lt == NLT - 1))
                        rec = exw.tile([CSEL, 1], F32, tag="rec")
                        nc.vector.reciprocal(rec[:], usum[:, 0:1])
                        updf = exw.tile([CSEL, D], FP16, tag="updf")
                        nc.vector.tensor_scalar(out=updf[:], in0=upd[:],
                                                scalar1=rec[:, :], scalar2=None,
                                                op0=ALU.mult)
                        nc.sync.dma_start(upd64d[h][:, :], updf[:, :])
                        offs40p = exw.tile([S, 1], U32, tag="offs40p")
                        spf = exw.tile([S, 1], F32, tag="spf")
                        nc.sync.dma_start(
                            spf[:], spd[h, 0:S].rearrange("(p o) -> p o", o=1))
                        nc.vector.tensor_copy(offs40p[:], spf[:])
                        upd40 = exw.tile([S, D], FP16, tag="upd40")
                        nc.gpsimd.indirect_dma_start(
                            out=upd40[:], out_offset=None, in_=upd64d[h][:, :],
                            in_offset=IOA(ap=offs40p[:, 0:1], axis=0))
                        nc.sync.dma_start(
                            exin[SEG * h:SEG * h + S * D]
                            .rearrange("(p d) -> p d", d=D), upd40[:, :])
                        slf32 = exw.tile([S, 1], F32, tag="slf32")
                        nc.sync.dma_start(
                            slf32[:], sld[h, 0:S].rearrange("(p o) -> p o", o=1))
                        slf = exw.tile([CSEL, 1], FP16, tag="slf")
                        nc.vector.memset(slf[:], 0.0)
                        nc.vector.tensor_copy(slf[0:S, :], slf32[:])
                        nc.sync.dma_start(
                            exin[SEG * h + S * D:SEG * h + S * D + CSEL]
                            .rearrange("(p o) -> p o", o=1), slf[:, :])
                        vrp = exps.tile([D, CSEL], F32, tag="psS")
                        for lt in range(NLT):
                            nc.tensor.matmul(
                                vrp[0:1, 0:D], ones_c2f[:, 0:1],
                                vall[:, 256 * lt + 64 * h:256 * lt + 64 * (h + 1)],
                                start=(lt == 0), stop=(lt == NLT - 1))
                        vrow = exw.tile([1, D], FP16, tag="vrow_s")
                        nc.scalar.activation(vrow[:], vrp[0:1, 0:D], AF.Copy,
                                             scale=1.0 / L)
                        nc.sync.dma_start(
                            exin[SEG * h + S * D + 64:SEG * h + S * D + 64 + D]
                            .rearrange("(o d) -> o d", o=1), vrow[0:1, :])

    # =================== P6: exchange + ctx build ===================
    post = ctx.enter_context(tc.tile_pool(name="post", bufs=1))
    # weight loads issued first: they overlap the collective's fixed latency
    wo_t = [post.tile([D, C], FP16, tag=f"wo{h8}", name=f"wo{h8}") for h8 in range(8)]
    for h8 in range(8):
        nc.sync.dma_start(wo_t[h8][:], Wo8[64 * h8:64 * (h8 + 1), :])
    w1_t = [post.tile([P, 2 * EXP * C], FP8, tag=f"w1_{p_}", name=f"w1_{p_}")
            for p_ in range(2)]
    w2_t = [post.tile([P, 2 * C], FP8, tag=f"w2_{pp}", name=f"w2_{pp}")
            for pp in range(8)]
    for p_ in range(2):
        nc.sync.dma_start(w1_t[p_][:], W1DR[p_][:, :])
    for pp in range(8):
        nc.sync.dma_start(w2_t[pp][:], W2DR[pp][:, :])
    nc.gpsimd.collective_compute(
        "AllGather", ALU.bypass,
        replica_groups=[[0, 1], [2, 3], [4, 5], [6, 7]],
        ins=[exin[:]], outs=[exout[:]])
    ctxT = [post.tile([D, HL], FP16, tag=f"ctxT{h8}", name=f"ctxT{h8}") for h8 in range(8)]
    vcol = [post.tile([D, 1], FP16, tag=f"vcol{h8}", name=f"vcol{h8}")
            for h8 in range(8)]
    with tc.tile_pool(name="cbw", bufs=3) as cbw:
        for h8 in range(8):
            base = SEG * h8
            nc.scalar.dma_start(vcol[h8][:, 0:1],
                                exout[base + S * D + 64:base + S * D + 64 + D]
                                .rearrange("(d o) -> d o", o=1))
            vbr = cbw.tile([S, D], FP16, tag="vbr")
            nc.scalar.dma_start(vbr[:],
                                exout[base + S * D + 64:base + S * D + 64 + D]
                                .rearrange("(o d) -> o d", o=1)
                                .to_broadcast([S, D]))
            lf = cbw.tile([S, 1], FP16, tag="lf")
            nc.sync.dma_start(lf[:], exout[base + S * D:base + S * D + S]
                              .rearrange("(p o) -> p o", o=1))
            lf32 = cbw.tile([S, 1], F32, tag="lf32")
            nc.vector.tensor_copy(lf32[:], lf[:])
            lsh = cbw.tile([S, 1], F32, tag="lsh")
            nc.vector.tensor_scalar(out=lsh[:], in0=lf32[:],
                                    scalar1=gsh_t[0:S, :], scalar2=None,
                                    op0=ALU.subtract)
            okm = cbw.tile([S, 1], U8, tag="okm")
            nc.vector.tensor_scalar(out=okm[:], in0=lsh[:], scalar1=-0.5,
                                    scalar2=None, op0=ALU.is_gt)
            big = cbw.tile([S, 1], F32, tag="big")
            nc.vector.memset(big[:], float(1 << 30))
            lok = cbw.tile([S, 1], F32, tag="lok")
            nc.vector.select(lok[:], okm[:], lsh[:], big[:])
            offs = cbw.tile([S, 1], U32, tag="offsx")
            nc.vector.tensor_copy(offs[:], lok[:])
            u40 = cbw.tile([S, D], FP16, tag="u40")
            nc.sync.dma_start(u40[:], exout[base:base + S * D]
                              .rearrange("(p d) -> p d", d=D))
            nc.vector.tensor_tensor(out=u40[:], in0=u40[:], in1=vbr[:],
                                    op=ALU.subtract)
            nc.gpsimd.indirect_dma_start(
                out=ctxd[h8][:, :],
                out_offset=IOA(ap=offs[:, 0:1], axis=0),
                in_=u40[:, :], in_offset=None,
                bounds_check=HL - 1, oob_is_err=False)
            nc.sync.dma_start_transpose(out=ctxT[h8][:], in_=ctxd[h8][:, :])

    # =================== P7: out-projection + residual ===================
    # rank-1 baseline: wvs[ct] = sum_h Wo_h^T vmean_h, added per-partition
    xT = [post.tile([P, HL], F32, tag=f"xT{ct}", name=f"xT{ct}") for ct in range(4)]
    wvs = [post.tile([P, 1], F32, tag=f"wvs{ct}", name=f"wvs{ct}") for ct in range(4)]
    with tc.tile_pool(name="p7w", bufs=2) as p7w, \
         tc.tile_pool(name="p7ps", bufs=2, space="PSUM") as p7ps:
        for ct in range(4):
            wvp = p7ps.tile([P, 1], F32, tag="wvp")
            for h8 in range(8):
                nc.tensor.matmul(wvp[:], wo_t[h8][:, 128 * ct:128 * (ct + 1)],
                                 vcol[h8][:, 0:1],
                                 start=(h8 == 0), stop=(h8 == 7))
            nc.scalar.copy(wvs[ct][:], wvp[:])
        for ct in range(4):
            ap_ = p7ps.tile([P, HL], F32, tag="attnT")
            for h8 in range(8):
                for nk in range(2):
                    nc.tensor.matmul(
                        ap_[:, 512 * nk:512 * (nk + 1)],
                        wo_t[h8][:, 128 * ct:128 * (ct + 1)],
                        ctxT[h8][:, 512 * nk:512 * (nk + 1)],
                        start=(h8 == 0), stop=(h8 == 7))
            qh = p7w.tile([P, HL], F32, tag="qh")
            nc.sync.dma_start(qh[:], qThalf[128 * ct:128 * (ct + 1), :])
            nc.vector.scalar_tensor_tensor(
                out=xT[ct][:], in0=ap_[:], scalar=wvs[ct][:, 0:1], in1=qh[:],
                op0=ALU.add, op1=ALU.add)

    # =================== P8: norm2 + FFN (fp8 DoubleRow) + residual ===========
    # t2d[p_] holds C-chunks (2p_, 2p_+1) as [P, (lq, two, 512)] fp8
    DR = mybir.MatmulPerfMode.DoubleRow
    t2d = [post.tile([P, 2 * HL], FP8, tag=f"t2d{p_}", name=f"t2d{p_}")
           for p_ in range(2)]
    with tc.tile_pool(name="p8w", bufs=2) as p8w, \
         tc.tile_pool(name="p8ps", bufs=2, space="PSUM") as p8ps, \
         tc.tile_pool(name="p8ps1", bufs=1, space="PSUM") as p8ps1:
        for ct in range(4):
            p_, two = ct // 2, ct % 2
            sig2 = p8w.tile([P, HL], F32, tag="sig2")
            nc.scalar.activation(sig2[:], xT[ct][:], AF.Sigmoid, scale=n2a_t[:, :])
            for lq in range(2):
                off = 1024 * lq + 512 * two
                nc.vector.scalar_tensor_tensor(
                    out=t2d[p_][:, off:off + 512], in0=sig2[:, 512 * lq:512 * (lq + 1)],
                    scalar=n2w_t[:, ct:ct + 1],
                    in1=xT[ct][:, 512 * lq:512 * (lq + 1)],
                    op0=ALU.mult, op1=ALU.mult)
        for lq in range(2):
            lsl = slice(512 * lq, 512 * (lq + 1))
            outp = [p8ps1.tile([P, 512], F32, tag=f"outp{cc}", name=f"outp{cc}") for cc in range(4)]
            for pp in range(8):
                hsd = p8w.tile([P, 1024], FP8, tag="hsd")
                for sub in range(2):
                    et = 2 * pp + sub
                    hp = p8ps.tile([P, 512], F32, tag="hp")
                    for p_ in range(2):
                        nc.tensor.matmul(
                            hp[:],
                            w1_t[p_][:, 256 * et:256 * (et + 1)]
                            .rearrange("p (two c) -> p two c", two=2),
                            t2d[p_][:, 1024 * lq:1024 * (lq + 1)]
                            .rearrange("p (two f) -> p two f", two=2),
                            start=(p_ == 0), stop=(p_ == 1), perf_mode=DR)
                    nc.scalar.activation(hsd[:, 512 * sub:512 * (sub + 1)],
                                         hp[:], AF.Relu)
                hr = hsd[:].rearrange("p (two f) -> p two f", two=2)
                for cc in range(4):
                    nc.tensor.matmul(
                        outp[cc][:],
                        w2_t[pp][:, 256 * cc:256 * (cc + 1)]
                        .rearrange("p (two c) -> p two c", two=2),
                        hr, start=(pp == 0), stop=(pp == 7), perf_mode=DR)
            for cc in range(4):
                ot = p8w.tile([P, 512], F32, tag="ot")
                nc.vector.scalar_tensor_tensor(
                    out=ot[:], in0=outp[cc][:], scalar=0.0, in1=xT[cc][:, lsl],
                    op0=ALU.add, op1=ALU.add)
                nc.sync.dma_start(oout[128 * cc:128 * (cc + 1), lsl], ot[:])
    ctx.close()


def build_nc(n_cores=8):
    nc = bacc.Bacc("TRN2", target_bir_lowering=False, debug=False,
                   num_devices=n_cores)
    with tile.TileContext(nc) as tc:
        _kernel(nc, tc)
    nc.compile()
    return nc


# ================= host side =================

def host_prep(query, sample_idx, Wq, Wk, Wv, Wo, W1, W2,
              n1_alpha, n1_w, n2_alpha, n2_w):
    f32 = np.float32
    mask = np.full((L, L), NEG, f32)
    mask[np.arange(L)[:, None], sample_idx] = 0.0
    maskb_h = mask.astype(ml_dtypes.float8_e5m2)
    A = np.zeros((L, L), f32)
    np.add.at(A, (np.repeat(np.arange(L), S), sample_idx.ravel()), 1.0)
    Ab_h = A.astype(ml_dtypes.float8_e4m3)
    iot16_h = (np.arange(128)[:, None] + 128 * np.arange(16)[None, :]).astype(f32)
    iot64_h = np.arange(CSEL, dtype=f32)
    ones2_h = np.concatenate([np.ones((128, 1), f32), np.zeros((128, 1), f32)], 1)
    WqTf = np.ascontiguousarray(Wq.T)
    WkTf = np.ascontiguousarray(Wk.T)
    WvTf = np.ascontiguousarray(Wv.T)
    Wo8_h = np.ascontiguousarray(Wo.T).astype(f16)
    fp8 = ml_dtypes.float8_e4m3
    W1Tf = np.ascontiguousarray(W1.T)   # [C, EXP*C]
    W2Tf = np.ascontiguousarray(W2.T)   # [EXP*C, C]
    # DoubleRow layouts: [128, (col-tile, two, 128)] pairing contraction chunks
    def dr_pack(WT, npair):
        # WT: [K, N] with K = 256*npair; returns list of [128, 2*N] fp8
        out = []
        for p_ in range(npair):
            a = WT[256 * p_:256 * p_ + 128, :]
            b = WT[256 * p_ + 128:256 * p_ + 256, :]
            # interleave per 128-col tile: [128, (nt, two, 128)]
            nt = WT.shape[1] // 128
            ar = a.reshape(128, nt, 128)
            br = b.reshape(128, nt, 128)
            st = np.stack([ar, br], axis=2)          # [128, nt, 2, 128]
            out.append(np.ascontiguousarray(
                st.reshape(128, 2 * WT.shape[1])).astype(fp8))
        return out
    W1dr = dr_pack(W1Tf, 2)
    W2dr = dr_pack(W2Tf, 8)
    ATdr = dr_pack(np.ascontiguousarray(A.T), 8)
    in_maps = []
    for c in range(8):
        b, g = c // 2, c % 2
        hsl = slice(256 * g, 256 * (g + 1))
        qT = np.ascontiguousarray(query[:, b, :].T)
        wqs = np.ascontiguousarray(WqTf[:, hsl])
        wks = np.ascontiguousarray(WkTf[:, hsl])
        wvs = np.ascontiguousarray(WvTf[:, hsl])
        wqh_ = wqs.astype(bfd)
        wql_ = (wqs - wqh_.astype(f32)).astype(bfd)
        wkh_ = wks.astype(bfd)
        wkl_ = (wks - wkh_.astype(f32)).astype(bfd)
        in_maps.append(dict(
            qTh=qT, qThalf=np.ascontiguousarray(qT[:, HL * g:HL * (g + 1)]),
            WqT=wqs, WkT=wks, WvT=wvs,
            WqTh=wqh_, WqTl=wql_, WkTh=wkh_, WkTl=wkl_,
            Wo8=Wo8_h,
            **{f"W1DR{p_}": W1dr[p_] for p_ in range(2)},
            **{f"W2DR{pp}": W2dr[pp] for pp in range(8)},
            maskb=maskb_h, Ab=Ab_h,
            **{f"ATDR{pp}": ATdr[pp] for pp in range(8)},
            n1a=np.full((128, 1), n1_alpha, f32),
            n1w=n1_w.reshape(C, 1).astype(f32),
            n2a=np.full((128, 1), n2_alpha, f32),
            n2w=n2_w.reshape(C, 1).astype(f32),
            iot16=iot16_h, iot64=iot64_h, ones2=ones2_h,
            identb=np.eye(128, dtype=np.float32).astype(bfd),
            gshift=np.full((128, 1), float(HL * g), f32),
        ))
    return in_maps


def assemble(results):
    out = np.empty((B, C, L), np.float32)
    for c in range(8):
        b, g = c // 2, c % 2
        out[b, :, HL * g:HL * (g + 1)] = results[c]["oout"]
    return np.ascontiguousarray(out.transpose(2, 0, 1))


def kernel(query, sample_idx, Wq, bq, Wk, bk, Wv, bv, Wo, bo,
           n1_alpha, n1_w, n1_b, n2_alpha, n2_w, n2_b, W1, b1, W2, b2,
           _trace=False):
    query = np.asarray(query, np.float32)
    sample_idx = np.asarray(sample_idx).astype(np.int64)
    for z in (bq, bk, bv, bo, n1_b, n2_b, b1, b2):
        assert np.all(np.asarray(z) == 0.0), "nonzero biases unsupported"
    in_maps = host_prep(query, sample_idx,
                        np.asarray(Wq, np.float32), np.asarray(Wk, np.float32),
                        np.asarray(Wv, np.float32), np.asarray(Wo, np.float32),
                        np.asarray(W1, np.float32), np.asarray(W2, np.float32),
                        float(np.asarray(n1_alpha).ravel()[0]),
                        np.asarray(n1_w, np.float32),
                        float(np.asarray(n2_alpha).ravel()[0]),
                        np.asarray(n2_w, np.float32))
    if "nc" not in _NC_CACHE:
        _NC_CACHE["nc"] = build_nc(8)
    res = run_bass_kernel_spmd(_NC_CACHE["nc"], in_maps, list(range(8)),
                               trace=_trace)
    out = assemble(res.results)
    if _trace:
        return out, res
    return out

